# revision 7
# baseline (speedup 1.0000x reference)
"""Multi-head attention on 8 Trainium2 NeuronCores.

Sharding: tensor-parallel over heads. 16 heads / 8 cores = 2 heads per
core. Each core reads the full x and computes Q/K/V for its 2 heads
(column slices of w_q/w_k/w_v), runs attention for those heads, and
multiplies by its row-slice of w_o, producing a partial [B, S, D]
output. The host sums the 8 partials and adds b_o + b_v @ w_o (the
V-bias contribution is linear and head-separable, so it is cheaper to
add once on the host than to apply b_v on-device).

On-device math (per core, per head h):
    x^T tiles produced by PE transposes (fp32, exact).
    Q^T = w_q_h^T x^T + b_q_h   [64, S]   (bias per-partition)
    K^T = w_k_h^T x^T + b_k_h   [64, S]
    V   = (w_v_h^T x^T)^T       [S, 64], ones column appended -> [S, 65]
    S^T = K^T-tile^T Q^T        [128 keys, q-chunk]  (keys on partitions)
    P~  = exp(S^T / 8)          (no max subtraction: logits are O(1))
    AV  = sum_kt Vtilde_kt^T P~_kt  -> [65, q]   rows 0-63 = attn^T
                                                 row 64    = denominator r
    out_h = (attn_h^T)^T w_o_h scaled per-token by 1/r_h (fused into the
            PSUM->SBUF copy as a per-partition scalar).
All matmuls run in fp32r (full PE rate at free-dim >= 256, ~1.6e-4 rel
err per matmul vs 2.4e-3 for bf16).
"""

import numpy as np

import concourse.bacc as bacc
import concourse.mybir as mybir
from concourse.tile import TileContext
from concourse.masks import make_identity
from concourse import bass_utils

dt = mybir.dt
F32 = dt.float32
F32R = dt.float32r
AF = mybir.ActivationFunctionType
ALU = mybir.AluOpType

B, S, D = 4, 2048, 1024
H, DH = 16, 64
NCORES = 8
HPC = H // NCORES          # heads per core = 2
DHC = HPC * DH             # 128 projection cols per core

_CACHE = {}


def build_nc(b=B, s=S):
    d = D
    n_tt = s // 128            # token tiles per batch
    n_kt = d // 128            # contraction tiles for projections
    qw = 1024 if s >= 1024 else s   # query-chunk width in attention
    n_qc = s // qw
    assert s % 512 == 0 and n_tt % 4 == 0 and d == 1024

    nc = bacc.Bacc("TRN2", target_bir_lowering=False, debug=False)

    x_d = nc.dram_tensor("x", [b, s, d], F32, kind="ExternalInput")
    wq_d = nc.dram_tensor("wq", [d, DHC], F32, kind="ExternalInput")
    wk_d = nc.dram_tensor("wk", [d, DHC], F32, kind="ExternalInput")
    wv_d = nc.dram_tensor("wv", [d, DHC], F32, kind="ExternalInput")
    bq_d = nc.dram_tensor("bq", [DHC, 1], F32, kind="ExternalInput")
    bk_d = nc.dram_tensor("bk", [DHC, 1], F32, kind="ExternalInput")
    wo_d = nc.dram_tensor("wo", [DHC, d], F32, kind="ExternalInput")
    out_d = nc.dram_tensor("out", [b, s, d], F32, kind="ExternalOutput")
    rs_d = nc.dram_tensor("rscratch", [b, HPC, s], F32)

    with TileContext(nc) as tc:
        with (
            tc.tile_pool(name="const", bufs=1) as cpool,
            tc.tile_pool(name="wpool", bufs=3 * n_kt) as wpool,
            tc.tile_pool(name="wstage", bufs=2) as wstage,
            tc.tile_pool(name="xa", bufs=5) as xa_pool,
            tc.tile_pool(name="xt", bufs=n_kt) as xt_pool,
            tc.tile_pool(name="qk", bufs=1) as qk_pool,
            tc.tile_pool(name="vt", bufs=3) as vt_pool,
            tc.tile_pool(name="at", bufs=1) as at_pool,
            tc.tile_pool(name="pexp", bufs=3) as pexp_pool,
            tc.tile_pool(name="rline", bufs=2) as rline_pool,
            tc.tile_pool(name="small", bufs=4) as small,
            tc.tile_pool(name="osb", bufs=3) as osb_pool,
            tc.tile_pool(name="ps", bufs=2, space="PSUM") as pp,
        ):
            # ---- constants & weights ----
            ident = cpool.tile([128, 128], F32, tag="ident")
            make_identity(nc, ident[:, :])
            ident_r = cpool.tile([128, 128], F32R, tag="ident_r")
            nc.vector.tensor_copy(ident_r[:, :], ident[:, :])

            ones_col = cpool.tile([128, 32], F32, tag="ones_col")
            nc.vector.memset(ones_col[:, :], 1.0)

            bq = cpool.tile([DHC, 1], F32, tag="bq")
            bk = cpool.tile([DHC, 1], F32, tag="bk")
            nc.sync.dma_start(out=bq[:, :], in_=bq_d[:, :])
            nc.sync.dma_start(out=bk[:, :], in_=bk_d[:, :])

            w_r = {}  # (name, kt) -> [128, DHC] f32r lhsT tiles
            for name, dram in (("q", wq_d), ("k", wk_d), ("v", wv_d)):
                for kt in range(n_kt):
                    stg = wstage.tile([128, DHC], F32, tag="wstg")
                    nc.sync.dma_start(
                        out=stg[:, :], in_=dram[kt * 128:(kt + 1) * 128, :]
                    )
                    wt = wpool.tile([128, DHC], F32R, tag="w")
                    nc.vector.tensor_copy(wt[:, :], stg[:, :])
                    w_r[(name, kt)] = wt
            wo_stg = cpool.tile([DHC, d], F32, tag="wo_stg")
            nc.sync.dma_start(out=wo_stg[:, :], in_=wo_d[:, :])
            wo = cpool.tile([DHC, d], F32R, tag="wo")
            nc.vector.tensor_copy(wo[:, :], wo_stg[:, :])

            for bi in range(b):
                # ---- stage A: x^T tiles for this batch ----
                # xT[kt] holds x[bi]^T rows kt*128..(kt+1)*128  -> [128, s]
                xT = [xt_pool.tile([128, s], F32R, tag="xt", name=f"xT{kt}")
                      for kt in range(n_kt)]
                for grp in range(n_tt // 4):
                    xas = []
                    for j in range(4):
                        tt = grp * 4 + j
                        xa = xa_pool.tile([128, d], F32, tag="xa")
                        nc.sync.dma_start(
                            out=xa[:, :],
                            in_=x_d[bi, tt * 128:(tt + 1) * 128, :],
                        )
                        xas.append(xa)
                    for kt in range(n_kt):
                        pst = pp.tile([128, 512], F32, tag="st")
                        for j in range(4):
                            nc.tensor.transpose(
                                pst[:, j * 128:(j + 1) * 128],
                                xas[j][:, kt * 128:(kt + 1) * 128],
                                ident[:, :],
                            )
                        nc.vector.tensor_copy(
                            xT[kt][:, grp * 512:(grp + 1) * 512], pst[:, :]
                        )

                # ---- stage B: projections ----
                qT = qk_pool.tile([DHC, s], F32R, tag="qT")
                kT = qk_pool.tile([DHC, s], F32R, tag="kT")
                for name, dst, bias in (("q", qT, bq), ("k", kT, bk)):
                    for c in range(s // 512):
                        ppr = pp.tile([128, 512], F32, tag="st")
                        for kt in range(n_kt):
                            nc.tensor.matmul(
                                ppr[:, :],
                                w_r[(name, kt)][:, :],
                                xT[kt][:, c * 512:(c + 1) * 512],
                                start=(kt == 0),
                                stop=(kt == n_kt - 1),
                            )
                        nc.vector.tensor_scalar_add(
                            dst[:, c * 512:(c + 1) * 512], ppr[:, :], bias[:, 0:1]
                        )
                # V^T then transpose to natural V with ones column
                vTn = qk_pool.tile([DHC, s], F32R, tag="vT")
                for c in range(s // 512):
                    ppr = pp.tile([128, 512], F32, tag="st")
                    for kt in range(n_kt):
                        nc.tensor.matmul(
                            ppr[:, :],
                            w_r[("v", kt)][:, :],
                            xT[kt][:, c * 512:(c + 1) * 512],
                            start=(kt == 0),
                            stop=(kt == n_kt - 1),
                        )
                    nc.vector.tensor_copy(vTn[:, c * 512:(c + 1) * 512], ppr[:, :])
                vt = [vt_pool.tile([128, n_tt * 65], F32R, tag="vt", name=f"vt{h}")
                      for h in range(HPC)]
                for h in range(HPC):
                    ones_dst = vt[h].rearrange(
                        "p (t c) -> p t c", c=65
                    )[:, :, 64]
                    nc.vector.tensor_copy(ones_dst, ones_col[:, 0:n_tt])
                for tt in range(n_tt):
                    ptv = pp.tile([128, 128], F32R, tag="av")
                    nc.tensor.transpose(
                        ptv[:, :],
                        vTn[:, tt * 128:(tt + 1) * 128],
                        ident_r[:, :],
                    )
                    for h in range(HPC):
                        nc.vector.tensor_copy(
                            vt[h][:, tt * 65:tt * 65 + 64],
                            ptv[:, h * 64:(h + 1) * 64],
                        )

                # ---- stage C: attention per head ----
                attnT = at_pool.tile([DHC, s], F32R, tag="attnT")
                rrec = [small.tile([128, n_tt], F32, tag="rrec", name=f"rrec{h}")
                        for h in range(HPC)]
                for h in range(HPC):
                    hs = slice(h * 64, (h + 1) * 64)
                    rline = rline_pool.tile([1, s], F32, tag="rline")
                    for qc in range(n_qc):
                        av = pp.tile([65, qw], F32, tag="av")
                        for kt in range(n_tt):
                            st = pp.tile([128, qw], F32, tag="st")
                            for j in range(qw // 512):
                                qs = slice(qc * qw + j * 512,
                                           qc * qw + (j + 1) * 512)
                                nc.tensor.matmul(
                                    st[:, j * 512:(j + 1) * 512],
                                    kT[hs, kt * 128:(kt + 1) * 128],
                                    qT[hs, qs],
                                    start=True,
                                    stop=True,
                                )
                            pexp = pexp_pool.tile([128, qw], F32R, tag="pexp")
                            nc.scalar.activation(
                                pexp[:, :], st[:, :], AF.Exp, scale=0.125
                            )
                            for j in range(qw // 512):
                                nc.tensor.matmul(
                                    av[:, j * 512:(j + 1) * 512],
                                    vt[h][:, kt * 65:(kt + 1) * 65],
                                    pexp[:, j * 512:(j + 1) * 512],
                                    start=(kt == 0),
                                    stop=(kt == n_tt - 1),
                                )
                        nc.vector.tensor_copy(
                            attnT[hs, qc * qw:(qc + 1) * qw], av[0:64, :]
                        )
                        nc.vector.tensor_copy(
                            rline[0:1, qc * qw:(qc + 1) * qw], av[64:65, :]
                        )
                    # 1/r as per-partition scalars: roundtrip via DRAM to
                    # reshape [1, s] -> [s//128, 128], then PE-transpose.
                    nc.sync.dma_start(
                        out=rs_d[bi, h, :].rearrange("(a c) -> a c", a=1),
                        in_=rline[0:1, :],
                    )
                    r16 = small.tile([n_tt, 128], F32, tag="r16")
                    nc.sync.dma_start(
                        out=r16[:, :],
                        in_=rs_d[bi, h, :].rearrange("(a c) -> a c", a=n_tt),
                    )
                    prt = pp.tile([128, n_tt], F32, tag="av")
                    nc.tensor.transpose(
                        prt[:, :], r16[:, :], ident[0:n_tt, 0:n_tt]
                    )
                    nc.vector.reciprocal(rrec[h][:, :], prt[:, :])

                # ---- stage D: output projection + normalize + combine ----
                for tt in range(n_tt):
                    for half in range(2):
                        cs = slice(half * 512, (half + 1) * 512)
                        po = []
                        for h in range(HPC):
                            hs = slice(h * 64, (h + 1) * 64)
                            p = pp.tile([128, 512], F32,
                                        tag="st" if h == 0 else "av")
                            nc.tensor.matmul(
                                p[:, :],
                                attnT[hs, tt * 128:(tt + 1) * 128],
                                wo[hs, cs],
                                start=True,
                                stop=True,
                                tile_position=(h * 64, 0),
                            )
                            po.append(p)
                        t1 = osb_pool.tile([128, 512], F32, tag="t1")
                        nc.vector.tensor_scalar_mul(
                            t1[:, :], po[0][:, :], rrec[0][:, tt:tt + 1]
                        )
                        osb = osb_pool.tile([128, 512], F32, tag="osb")
                        nc.vector.scalar_tensor_tensor(
                            osb[:, :],
                            po[1][:, :],
                            rrec[1][:, tt:tt + 1],
                            t1[:, :],
                            ALU.mult,
                            ALU.add,
                        )
                        nc.sync.dma_start(
                            out=out_d[bi, tt * 128:(tt + 1) * 128, cs],
                            in_=osb[:, :],
                        )

    nc.compile()
    return nc


def _get_nc(b, s):
    key = (b, s)
    if key not in _CACHE:
        _CACHE[key] = build_nc(b, s)
    return _CACHE[key]


def make_in_maps(x, w_q, b_q, w_k, b_k, w_v, w_o):
    in_maps = []
    for i in range(NCORES):
        cs = slice(i * DHC, (i + 1) * DHC)
        in_maps.append({
            "x": np.ascontiguousarray(x, dtype=np.float32),
            "wq": np.ascontiguousarray(w_q[:, cs], dtype=np.float32),
            "wk": np.ascontiguousarray(w_k[:, cs], dtype=np.float32),
            "wv": np.ascontiguousarray(w_v[:, cs], dtype=np.float32),
            "bq": np.ascontiguousarray(b_q[cs, None], dtype=np.float32),
            "bk": np.ascontiguousarray(b_k[cs, None], dtype=np.float32),
            "wo": np.ascontiguousarray(w_o[cs, :], dtype=np.float32),
        })
    return in_maps


def kernel(x, w_q, b_q, w_k, b_k, w_v, b_v, w_o, b_o, _trace=False):
    x = np.asarray(x, dtype=np.float32)
    nc = _get_nc(x.shape[0], x.shape[1])
    in_maps = make_in_maps(x, w_q, b_q, w_k, b_k, w_v, w_o)
    kw = {}
    if _trace:
        import tempfile
        kw = dict(trace=True, trace_cores=list(range(NCORES)),
                  tmpdir=tempfile.mkdtemp(prefix="mha_trace_"))
    res = bass_utils.run_bass_kernel_spmd(
        nc, in_maps, core_ids=list(range(NCORES)), **kw
    )
    out = np.zeros(x.shape, dtype=np.float32)
    for i in range(NCORES):
        out += np.asarray(res.results[i]["out"])
    out += np.asarray(b_o, dtype=np.float32)[None, None, :]
    out += (np.asarray(b_v, dtype=np.float32)
            @ np.asarray(w_o, dtype=np.float32))[None, None, :]
    if _trace:
        return out, res
    return out


# revision 8
# speedup vs baseline: 1.0596x; 1.0596x over previous
"""Multi-head attention on 8 Trainium2 NeuronCores — fp16 pipeline.

Sharding: tensor-parallel over heads (2 heads/core), full batch on every
core; host sums the 8 partial outputs and adds b_o + b_v @ w_o.

vs the fp32r version: all matmul operands are fp16 (1 cycle/row, FWL
fast weight loads, 1024-wide moving operand, 2-byte DMA-xbar transpose
for x^T). fp32r matmuls are self-loading (one serialized ~208 ns
LDWEIGHTS per matmul) which caps them at ~2x slower in practice.

Per core, per batch bi:
  A: xT[kt] [128, S] fp16 <- DMA-xbar-transpose of x16[bi, :, kt*128:...]
  B: Q^T/K^T = w^T xT + b (PSUM f32, bias fused in DVE copy -> fp16)
     V natural [tok, 128] via lhsT = xT tile, rhs = w_v tile; DVE
     strided-copy into vt chunks [V_A | 1 | V_B | 1] per 130 cols.
  C: per head, per 1024-wide q-chunk: 17-step software pipeline
     kt:   S^T MM [128k, 1024q] -> PSUM   (lhsT = K^T tile [64, 128])
           stage-D fill for batch bi-1 (2 MMs + 2 DVE + DMA per unit)
     kt-1: ACT exp(PSUM * 0.125) -> pexp fp16
           AV MM [65, 1024] accumulate (lhsT = vt chunk [128, 65])
     row 64 of AV = softmax denominators r; 1/r via PE transpose +
     DVE reciprocal -> per-partition scalars [128, n_tt].
  D (interleaved into C of batch bi+1): per (tt, half):
     po_h [128, 512] = attnT_h^T @ w_o_h; DVE: t1 = po_0 * rrec0;
     osb = (po_1 * rrec1) + t1 -> fp16 -> DMA out.
"""

import numpy as np

import concourse.bacc as bacc
import concourse.mybir as mybir
from concourse.tile import TileContext
from concourse.masks import make_identity
from concourse import bass_utils

dt = mybir.dt
F32 = dt.float32
F16 = dt.float16
AF = mybir.ActivationFunctionType
ALU = mybir.AluOpType

B, S, D = 4, 2048, 1024
H, DH = 16, 64
NCORES = 8
HPC = H // NCORES          # heads per core = 2
DHC = HPC * DH             # 128 projection cols per core

_CACHE = {}


def build_nc(b=B, s=S):
    d = D
    n_tt = s // 128            # token tiles per batch
    n_kt = d // 128            # contraction tiles for projections
    qw = 1024 if s >= 1024 else s
    n_jc = s // qw
    assert s % 512 == 0 and d == 1024

    nc = bacc.Bacc("TRN2", target_bir_lowering=False, debug=False)

    x_d = nc.dram_tensor("x", [b, s, d], F16, kind="ExternalInput")
    wq_d = nc.dram_tensor("wq", [d, DHC], F16, kind="ExternalInput")
    wk_d = nc.dram_tensor("wk", [d, DHC], F16, kind="ExternalInput")
    wv_d = nc.dram_tensor("wv", [d, DHC], F16, kind="ExternalInput")
    bq_d = nc.dram_tensor("bq", [DHC, 1], F32, kind="ExternalInput")
    bk_d = nc.dram_tensor("bk", [DHC, 1], F32, kind="ExternalInput")
    wo_d = nc.dram_tensor("wo", [DHC, d], F16, kind="ExternalInput")
    out_d = nc.dram_tensor("out", [b, s, d], F16, kind="ExternalOutput")
    rs_d = nc.dram_tensor("rscratch", [b, HPC, s], F32)

    with TileContext(nc) as tc:
        with (
            tc.tile_pool(name="const", bufs=1) as cpool,
            tc.tile_pool(name="wpool", bufs=3 * n_kt) as wpool,
            tc.tile_pool(name="xt", bufs=2 * n_kt) as xt_pool,
            tc.tile_pool(name="qk", bufs=2) as qk_pool,
            tc.tile_pool(name="vt", bufs=3) as vt_pool,
            tc.tile_pool(name="at", bufs=2) as at_pool,
            tc.tile_pool(name="pexp", bufs=3) as pexp_pool,
            tc.tile_pool(name="rline", bufs=2) as rline_pool,
            tc.tile_pool(name="small", bufs=6) as small,
            tc.tile_pool(name="osb", bufs=4) as osb_pool,
            tc.tile_pool(name="ps", bufs=1, space="PSUM") as pp,
        ):
            # ---- constants & weights ----
            ident = cpool.tile([128, 128], F32, tag="ident")
            make_identity(nc, ident[:, :])
            ones_col = cpool.tile([128, 32], F16, tag="ones_col")
            nc.vector.memset(ones_col[:, :], 1.0)

            bq = cpool.tile([DHC, 1], F32, tag="bq")
            bk = cpool.tile([DHC, 1], F32, tag="bk")
            nc.sync.dma_start(out=bq[:, :], in_=bq_d[:, :])
            nc.sync.dma_start(out=bk[:, :], in_=bk_d[:, :])

            w16 = {}
            for name, dram in (("q", wq_d), ("k", wk_d), ("v", wv_d)):
                for kt in range(n_kt):
                    wt = wpool.tile([128, DHC], F16, tag="w",
                                    name=f"w_{name}{kt}")
                    nc.sync.dma_start(
                        out=wt[:, :], in_=dram[kt * 128:(kt + 1) * 128, :]
                    )
                    w16[(name, kt)] = wt
            wo = cpool.tile([DHC, d], F16, tag="wo")
            nc.sync.dma_start(out=wo[:, :], in_=wo_d[:, :])

            # stage-D state carried across the batch loop
            prev_d = None   # (attnT, rrec) of previous batch
            d_queue = []    # pending stage-D units for prev batch

            def emit_d_unit(bi_out):
                """Emit one (tt, half) output unit of the previous batch."""
                if not d_queue:
                    return
                attnT_p, rrec_p, tt, half = d_queue.pop(0)
                cs = slice(half * 512, (half + 1) * 512)
                poA = pp.tile([128, 512], F32, tag="poA", name="poA")
                poB = pp.tile([128, 512], F32, tag="poB", name="poB")
                nc.tensor.matmul(
                    poA[:, :], attnT_p[0:64, tt * 128:(tt + 1) * 128],
                    wo[0:64, cs], start=True, stop=True,
                    tile_position=(0, 0),
                )
                nc.tensor.matmul(
                    poB[:, :], attnT_p[64:128, tt * 128:(tt + 1) * 128],
                    wo[64:128, cs], start=True, stop=True,
                    tile_position=(64, 0),
                )
                t1 = osb_pool.tile([128, 512], F32, tag="t1", name="t1")
                nc.vector.tensor_scalar_mul(
                    t1[:, :], poA[:, :], rrec_p[0][:, tt:tt + 1]
                )
                osb = osb_pool.tile([128, 512], F16, tag="osb", name="osb")
                nc.vector.scalar_tensor_tensor(
                    osb[:, :], poB[:, :], rrec_p[1][:, tt:tt + 1], t1[:, :],
                    ALU.mult, ALU.add,
                )
                nc.sync.dma_start(
                    out=out_d[bi_out, tt * 128:(tt + 1) * 128, cs],
                    in_=osb[:, :],
                )

            for bi in range(b):
                # ---- stage A: x^T via DMA xbar transpose ----
                xT = [xt_pool.tile([128, s], F16, tag="xt", name=f"xT{kt}")
                      for kt in range(n_kt)]
                for kt in range(n_kt):
                    nc.sync.dma_start_transpose(
                        xT[kt][:, :], x_d[bi, :, kt * 128:(kt + 1) * 128]
                    )

                # ---- stage B: Q^T, K^T projections ----
                qT = qk_pool.tile([DHC, s], F16, tag="qT")
                kT = qk_pool.tile([DHC, s], F16, tag="kT")
                for name, dst, bias in (("q", qT, bq), ("k", kT, bk)):
                    for c in range(s // qw):
                        ppr = pp.tile([128, qw], F32, tag="st", bufs=2,
                                      name="ppr")
                        for kt in range(n_kt):
                            for j in range(qw // 512):
                                nc.tensor.matmul(
                                    ppr[:, j * 512:(j + 1) * 512],
                                    w16[(name, kt)][:, :],
                                    xT[kt][:, c * qw + j * 512:
                                            c * qw + (j + 1) * 512],
                                    start=(kt == 0),
                                    stop=(kt == n_kt - 1),
                                )
                        nc.vector.tensor_scalar_add(
                            dst[:, c * qw:(c + 1) * qw], ppr[:, :], bias[:, 0:1]
                        )
                # V natural, interleaved-head layout [V_A |1| V_B |1] per 130
                vt = vt_pool.tile([128, n_tt * 130], F16, tag="vt")
                ones_dst = vt.rearrange("p (t two sv) -> p t two sv",
                                        two=2, sv=65)[:, :, :, 64]
                nc.vector.tensor_copy(ones_dst, ones_col[:, 0:2 * n_tt]
                                      .rearrange("p (t two) -> p t two", two=2))
                for tt in range(n_tt):
                    pv = pp.tile([128, 128], F32, tag="poA", name="pv")
                    for kt in range(n_kt):
                        nc.tensor.matmul(
                            pv[:, :],
                            xT[kt][:, tt * 128:(tt + 1) * 128],
                            w16[("v", kt)][:, :],
                            start=(kt == 0),
                            stop=(kt == n_kt - 1),
                        )
                    vdst = vt.rearrange("p (t two sv) -> p t two sv",
                                        two=2, sv=65)[:, tt, :, 0:64]
                    nc.vector.tensor_copy(
                        vdst, pv.rearrange("p (two sv) -> p two sv", two=2)
                    )

                # ---- stage C: attention (+ interleaved stage D of bi-1) ----
                attnT = at_pool.tile([DHC, s], F16, tag="attnT")
                rrec = [small.tile([128, n_tt], F32, tag="rrec",
                                   name=f"rrec{h}") for h in range(HPC)]
                vtv = vt.rearrange("p (t two sv) -> p t two sv", two=2, sv=65)
                for h in range(HPC):
                    hs = slice(h * 64, (h + 1) * 64)
                    rline = rline_pool.tile([1, s], F32, tag="rline")
                    for jc in range(n_jc):
                        qs = slice(jc * qw, (jc + 1) * qw)
                        av = pp.tile([65, qw], F32, tag="av", name="av")
                        pexps = {}
                        for kt in range(n_tt + 1):
                            if kt < n_tt:
                                st = pp.tile([128, qw], F32, tag="st",
                                             bufs=2, name="st")
                                for j in range(qw // 512):
                                    nc.tensor.matmul(
                                        st[:, j * 512:(j + 1) * 512],
                                        kT[hs, kt * 128:(kt + 1) * 128],
                                        qT[hs, jc * qw + j * 512:
                                           jc * qw + (j + 1) * 512],
                                        start=True, stop=True,
                                    )
                                emit_d_unit(bi - 1)
                                pexp = pexp_pool.tile([128, qw], F16,
                                                      tag="pexp", name="pexp")
                                nc.scalar.activation(
                                    pexp[:, :], st[:, :], AF.Exp, scale=0.125
                                )
                                pexps[kt] = pexp
                            if kt > 0:
                                px = pexps.pop(kt - 1)
                                for j in range(qw // 512):
                                    nc.tensor.matmul(
                                        av[:, j * 512:(j + 1) * 512],
                                        vtv[:, kt - 1, h, :],
                                        px[:, j * 512:(j + 1) * 512],
                                        start=(kt == 1),
                                        stop=(kt == n_tt),
                                    )
                        nc.vector.tensor_copy(attnT[hs, qs], av[0:64, :])
                        nc.vector.tensor_copy(
                            rline[0:1, qs], av[64:65, :]
                        )
                    # 1/r -> per-partition scalars via DRAM roundtrip
                    nc.sync.dma_start(
                        out=rs_d[bi, h, :].rearrange("(a c) -> a c", a=1),
                        in_=rline[0:1, :],
                    )
                    r16 = small.tile([n_tt, 128], F32, tag="r16")
                    nc.sync.dma_start(
                        out=r16[:, :],
                        in_=rs_d[bi, h, :].rearrange("(a c) -> a c", a=n_tt),
                    )
                    prt = pp.tile([128, n_tt], F32, tag="av", name="prt")
                    nc.tensor.transpose(
                        prt[:, :], r16[:, :], ident[0:n_tt, 0:n_tt]
                    )
                    nc.vector.reciprocal(rrec[h][:, :], prt[:, :])

                # queue stage D for this batch
                prev_d = (attnT, rrec)
                for tt in range(n_tt):
                    for half in range(2):
                        d_queue.append((attnT, rrec, tt, half))

            # flush the last batch's stage D
            while d_queue:
                emit_d_unit(b - 1)

    nc.compile()
    return nc


def _get_nc(b, s):
    key = (b, s)
    if key not in _CACHE:
        _CACHE[key] = build_nc(b, s)
    return _CACHE[key]


def make_in_maps(x, w_q, b_q, w_k, b_k, w_v, w_o):
    x16 = np.ascontiguousarray(x, dtype=np.float16)
    wq16 = np.asarray(w_q, dtype=np.float16)
    wk16 = np.asarray(w_k, dtype=np.float16)
    wv16 = np.asarray(w_v, dtype=np.float16)
    wo16 = np.asarray(w_o, dtype=np.float16)
    in_maps = []
    for i in range(NCORES):
        cs = slice(i * DHC, (i + 1) * DHC)
        in_maps.append({
            "x": x16,
            "wq": np.ascontiguousarray(wq16[:, cs]),
            "wk": np.ascontiguousarray(wk16[:, cs]),
            "wv": np.ascontiguousarray(wv16[:, cs]),
            "bq": np.ascontiguousarray(b_q[cs, None], dtype=np.float32),
            "bk": np.ascontiguousarray(b_k[cs, None], dtype=np.float32),
            "wo": np.ascontiguousarray(wo16[cs, :]),
        })
    return in_maps


def kernel(x, w_q, b_q, w_k, b_k, w_v, b_v, w_o, b_o, _trace=False):
    x = np.asarray(x, dtype=np.float32)
    nc = _get_nc(x.shape[0], x.shape[1])
    in_maps = make_in_maps(x, w_q, b_q, w_k, b_k, w_v, w_o)
    kw = {}
    if _trace:
        import tempfile
        kw = dict(trace=True, trace_cores=list(range(NCORES)),
                  tmpdir=tempfile.mkdtemp(prefix="mha_trace_"))
    res = bass_utils.run_bass_kernel_spmd(
        nc, in_maps, core_ids=list(range(NCORES)), **kw
    )
    out = np.zeros(x.shape, dtype=np.float32)
    for i in range(NCORES):
        out += np.asarray(res.results[i]["out"], dtype=np.float32)
    out += np.asarray(b_o, dtype=np.float32)[None, None, :]
    out += (np.asarray(b_v, dtype=np.float32)
            @ np.asarray(w_o, dtype=np.float32))[None, None, :]
    if _trace:
        return out, res
    return out
